# revision 32
# baseline (speedup 1.0000x reference)
"""BERT self-attention (B=4, S=2048, H=768, 12 heads x d=64) on 8 Trainium2
NeuronCores.

Sharding: core c handles batch b = c//2 and head group hg = c%2 (6 heads).
Each core computes q/k/v projections for its 6 heads from its batch's
hidden_states, then attention per head. No cross-core communication; the host
scatters inputs and gathers/reassembles the output.

Per-core layouts (SBUF is [128 partitions x free]):
  xT   [128, KC, 2048]  x[b].T in bf16; when any bias is nonzero a row of
                        ones is appended (KC=7) to fold the bias in, else
                        KC=6 (saves 1/7 of projection matmul time)
  w    [128, KC, 1152]  packed weight column-slices in consumption order
                        [q-pair0|k-pair0 | q-p1|k-p1 | q-p2|k-p2 | v(384)]
                        (+ bias row when KC=7), bf16 — the pair-0 blocks DMA
                        first so the first projections start ~4us in
  qT/kT [128, 3, 2048]  per head-pair stacked d-dims (head even: p0-63,
                        odd: p64-127), bf16
  v    [128, 16, 6, 128] token-major v per j-tile; cols 0-63 of each head
                        hold em[j] (=exp(mask_j), ones for a zero mask), cols
                        64-127 the (em-scaled) v data: the ctx matmul then
                        leaves 64 copies of the masked sumexp in psum rows
                        0-63 (reciprocal_approx_fast requires base partition
                        0) and ctx^T in rows 64-127; the additive mask costs
                        no per-score work at all
  scoresT psum [128 j, 2 heads, 512 i] -> exp (ACT engine) -> ex sbuf bf16
  ctx^T psum [128, 512]: rows 0-63 masked-sumexp copies, rows 64-127 ctx^T.

The two heads of a pair run their K=64 scores matmuls in disjoint PE row
groups (partitions 0-63 / 64-127) concurrently (verified in the profile:
paired MATMULs start ~4ns apart with equal durations).

The ACT engine's exp (192 instrs x ~1us issue) and the PE (~170us of matmul)
are nearly balanced, so emission interleaves them at j-tile granularity:
within a chunk, j-tile jt emits scores(jt) then both heads' ctx-accumulate
of jt-CSH of the same chunk; the previous chunk's last CSH ctx tiles and its
finalize land in j-tiles 0..CSH-1 of the current chunk, so the 2-deep ctx
psum ring never stalls the PE.  Projection work is spread across the early
chunks in half-group (6-matmul) closures so no single fill delays the next
score tile by more than ~1.3us.  A short burst of dummy matmuls on a scratch
tile runs during the initial DMA wait so the PE_HAM clock gate (1.2 ->
2.4 GHz after ~3.4us of sustained activity) is warm when real work starts.
"""
import os

import numpy as np

if not os.environ.get("KERNEL_TRACE"):
    # a stray BASS_TRACE in the environment would send run_bass_kernel_spmd
    # down a profiling path that needs hooks this container may not have
    os.environ.setdefault("BASS_NEVER_TRACE", "1")

import ml_dtypes

import concourse.bass as bass
import concourse.mybir as mybir
import concourse.tile as tile
from concourse import bacc
from concourse.bass import ts
from concourse.bass_utils import run_bass_kernel_spmd

F32 = mybir.dt.float32
BF16 = mybir.dt.bfloat16

HIDDEN = 768
N_HEADS = 12
HEAD_DIM = 64
B = 4
S = 2048
HPC = 6          # heads per core
NI = S // 512    # 4 i-chunks of 512
NJ = S // 128    # 16 j-tiles of 128
VW = 128         # em copies (64) | v (64): full-partition ctx psum
CSH = 6          # ctx interleave shift (j-tiles of runway behind exp)
WCOL = 6 * 128 + HPC * HEAD_DIM  # packed weight columns (qk pairs + v)

_cache = {}
last_results = None


def _build(kc):
    nc = bacc.Bacc("TRN2", target_bir_lowering=False, debug=False, num_devices=8)

    xT_d = nc.dram_tensor("xT", [kc * 128, S], BF16, kind="ExternalInput")
    wq_d = nc.dram_tensor("wq", [kc * 128, HPC * HEAD_DIM], BF16, kind="ExternalInput")
    wk_d = nc.dram_tensor("wk", [kc * 128, HPC * HEAD_DIM], BF16, kind="ExternalInput")
    wv_d = nc.dram_tensor("wv", [kc * 128, HPC * HEAD_DIM], BF16, kind="ExternalInput")
    em_d = nc.dram_tensor("em", [128, NJ], F32, kind="ExternalInput")
    out_d = nc.dram_tensor("out", [HPC, HEAD_DIM, S], BF16, kind="ExternalOutput")

    with tile.TileContext(nc) as tc:
        with (
            tc.tile_pool(name="const", bufs=1) as cpool,
            tc.tile_pool(name="qk", bufs=1) as qkpool,
            tc.tile_pool(name="vp", bufs=1) as vpool,
            tc.tile_pool(name="op", bufs=3) as opool,
            tc.tile_pool(name="rp", bufs=2) as rpool,
            tc.tile_pool(name="xw", bufs=1) as xwpool,
            tc.tile_pool(name="ex", bufs=8) as expool,
            tc.tile_pool(name="pss", bufs=3, space="PSUM") as pss,
            tc.tile_pool(name="psc", bufs=2, space="PSUM") as psc,
        ):
            em = cpool.tile([128, NJ], F32)
            nc.sync.dma_start(em[:], em_d[:])

            # HAM warm-up: dummy matmuls on a scratch tile with no DMA deps
            # keep the PE busy through the initial input-DMA window
            scr = cpool.tile([128, 512], BF16)
            nc.vector.memset(scr[:], 0.5)
            wps = psc.tile([128, 512], F32, tag="c", name="warm")
            for i in range(20):
                nc.tensor.matmul(wps[:], scr[:, 0:128], scr[:],
                                 start=(i == 0), stop=(i == 19))

            qT = qkpool.tile([128, HPC // 2, S], BF16)
            kT = qkpool.tile([128, HPC // 2, S], BF16)
            v = vpool.tile([128, NJ, HPC, VW], BF16)

            xT = xwpool.tile([128, kc, S], BF16)
            wq = xwpool.tile([128, kc, HPC * HEAD_DIM], BF16)
            wk = xwpool.tile([128, kc, HPC * HEAD_DIM], BF16)
            wv = xwpool.tile([128, kc, HPC * HEAD_DIM], BF16)
            # weights first, then xT in token-slice-outer order (1024-wide
            # slices keep 2KB partition lines for full DMA throughput); the
            # loads round-robin over four engines' DMA rings so several DMA
            # engines stream in parallel
            dmae = [nc.sync, nc.gpsimd, nc.scalar]
            nd = len(dmae)
            di = 0
            for c in range(kc):
                dmae[di % nd].dma_start(wq[:, c, :], wq_d[ts(c, 128), :]); di += 1
                dmae[di % nd].dma_start(wk[:, c, :], wk_d[ts(c, 128), :]); di += 1
            for h2 in range(2):
                for c in range(kc):
                    dmae[di % nd].dma_start(
                        xT[:, c, ts(h2, 1024)], xT_d[ts(c, 128), ts(h2, 1024)]
                    )
                    di += 1
            for c in range(kc):
                dmae[di % nd].dma_start(wv[:, c, :], wv_d[ts(c, 128), :])
                di += 1

            def emit_qk(p, which=(0, 1), halves=(0, 1)):
                # c-outer within each half: the stationary weight chunk is
                # loaded once per 2 token-chunk matmuls (2-bank psum groups)
                for w_, dst in [((wq, qT), (wk, kT))[w] for w in which]:
                    for half in halves:
                        acc = pss.tile([128, 2, 512], F32, tag="s")
                        for c in range(kc):
                            for n2 in range(2):
                                n = 2 * half + n2
                                nc.tensor.matmul(
                                    acc[:, n2, :], w_[:, c, ts(p, 128)],
                                    xT[:, c, ts(n, 512)],
                                    start=(c == 0), stop=(c == kc - 1),
                                )
                        nc.vector.tensor_copy(
                            dst[:, p, ts(half, 1024)],
                            acc[:].rearrange("p a n -> p (a n)"),
                        )

            def emit_qk_n(p, wk_, n):
                # one self-contained half projection group: 6 matmuls into a
                # 1-bank psum tile + drain; small enough (~1.3us) not to
                # starve the ACT engine when used as an in-chunk fill
                w_, dst = ((wq, qT), (wk, kT))[wk_]
                acc = pss.tile([128, 512], F32, tag="s", name="qa")
                for c in range(kc):
                    nc.tensor.matmul(
                        acc[:], w_[:, c, ts(p, 128)], xT[:, c, ts(n, 512)],
                        start=(c == 0), stop=(c == kc - 1),
                    )
                nc.vector.tensor_copy(dst[:, p, ts(n, 512)], acc[:])

            def emit_v(jt):
                # v projection for one j-tile: psum [128 tokens, 384]; em
                # (exp additive mask, ones when maskless) folds in during
                # the psum drain; the sumexp columns are written as em
                # directly (0*pv + em), so no separate memset is needed
                pv = pss.tile([128, HPC * HEAD_DIM], F32, tag="s")
                for c in range(kc):
                    nc.tensor.matmul(
                        pv[:], xT[:, c, ts(jt, 128)], wv[:, c, :],
                        start=(c == 0), stop=(c == kc - 1),
                    )
                pvh = pv[:].rearrange("p (h e) -> p h e", h=HPC)
                nc.vector.tensor_scalar_mul(
                    v[:, jt, :, HEAD_DIM:VW], pvh, em[:, jt:jt + 1],
                )
                nc.vector.tensor_scalar(
                    v[:, jt, :, 0:HEAD_DIM], pvh, 0.0, em[:, jt:jt + 1],
                    mybir.AluOpType.mult, mybir.AluOpType.add,
                )

            def emit_ctx(pcs, pr_, exs, jt):
                pex = exs[jt // (NJ // 4)]
                for a_ in range(2):
                    nc.tensor.matmul(
                        pcs[a_][:], v[:, jt, 2 * pr_ + a_, :],
                        pex[:, jt % (NJ // 4), a_, :],
                        start=(jt == 0), stop=False,
                    )

            def emit_fin_head(pr_, ic, a_, pc):
                # rows 0-63 are 64 copies of the masked sumexp
                rc = rpool.tile([64, 512], F32, tag="rc")
                nc.vector.reciprocal_approx_fast(rc[:], pc[0:64, :])
                o = opool.tile([64, 512], BF16, tag="o")
                nc.vector.tensor_tensor(
                    o[:], pc[64:VW, :], rc[:], op=mybir.AluOpType.mult
                )
                nc.sync.dma_start(out_d[2 * pr_ + a_, :, ts(ic, 512)], o[:])

            def emit_tail(prev, jt):
                """trailing ctx j-tile NJ-CSH+jt of the previous chunk, then
                both heads' finalize once the last accumulation lands."""
                (pr_, ic), exs, pcs = prev
                cjt = NJ - CSH + jt
                pex = exs[cjt // (NJ // 4)]
                for a_ in range(2):
                    nc.tensor.matmul(
                        pcs[a_][:], v[:, cjt, 2 * pr_ + a_, :],
                        pex[:, cjt % (NJ // 4), a_, :],
                        start=False, stop=(cjt == NJ - 1),
                    )
                if jt == CSH - 1:
                    for a_ in range(2):
                        emit_fin_head(pr_, ic, a_, pcs[a_])

            def emit_chunk(ch, prev, fills):
                """scores+exp of chunk `ch`; ctx of j-tile jt-CSH interleaves
                after scores of jt; the previous chunk's last CSH ctx tiles
                and finalize land in j-tiles 0..CSH-1; `fills` consumed one
                per j-tile."""
                pr_, ic = ch
                exs = []
                pcs = [psc.tile([128, 512], F32, tag="c", name=f"pc{a_}")
                       for a_ in range(2)]
                pend = []
                sstiles = {}

                def emit_scores(jt):
                    ss = pss.tile([128, 2, 512], F32, tag="s", name="ss")
                    sstiles[jt] = ss
                    for a_ in range(2):
                        po = 64 * a_
                        nc.tensor.matmul(
                            ss[:, a_, :],
                            kT[po:po + 64, pr_, ts(jt, 128)],
                            qT[po:po + 64, pr_, ts(ic, 512)],
                            start=True, stop=True,
                        )

                # scores run two j-tiles ahead of their exp so a fill lump
                # never sits between the ACT engine and its next input
                emit_scores(0)
                emit_scores(1)
                for jt in range(NJ):
                    if jt % (NJ // 4) == 0:
                        ex = expool.tile([128, NJ // 4, 2, 512], BF16, tag="e")
                        exs.append(ex)
                    nc.scalar.activation(
                        ex[:, jt % (NJ // 4), :, :], sstiles.pop(jt)[:],
                        mybir.ActivationFunctionType.Exp,
                        scale=1.0 / np.sqrt(HEAD_DIM),
                    )
                    if jt + 2 < NJ:
                        emit_scores(jt + 2)
                    if jt < len(fills):
                        # fills go before the tail: a late v fill in early
                        # j-tiles must precede the previous chunk's tail
                        # ctx that reads it
                        fills[jt]()
                    if prev is not None and jt < CSH:
                        emit_tail(prev, jt)
                    if jt >= CSH:
                        pend.append(jt - CSH)
                    if jt >= len(fills):
                        # ctx defers out of fill-carrying slots so no slot
                        # exceeds the ACT engine's ~1us exp cadence; drain
                        # up to two pending ctx j-tiles per free slot
                        for _ in range(3):
                            if pend:
                                emit_ctx(pcs, pr_, exs, pend.pop(0))
                while pend:
                    emit_ctx(pcs, pr_, exs, pend.pop(0))
                return (ch, exs, pcs)

            # prologue: only what chunk 0 needs (k-pair0 fully, q-pair0
            # half 0); the rest of the projections spread into the chunks
            emit_qk(0, (1,), (0,))
            emit_qk(0, (0,), (0,))
            emit_qk(0, (1,), (1,))

            chunks = [(p_, i_) for p_ in range(HPC // 2) for i_ in range(NI)]
            fill = {
                0: [lambda jt=jt: emit_v(jt) for jt in range(10)],
                1: ([lambda jt=jt: emit_v(jt) for jt in range(10, NJ)]
                    + [lambda n=n: emit_qk_n(0, 0, n) for n in (2, 3)]),
                2: [lambda n=n: emit_qk_n(1, 1, n) for n in range(4)],
                3: [lambda n=n: emit_qk_n(1, 0, n) for n in (0, 1)],
                5: [lambda n=n: emit_qk_n(1, 0, n) for n in (2, 3)],
                6: [lambda n=n: emit_qk_n(2, 1, n) for n in range(4)],
                7: [lambda n=n: emit_qk_n(2, 0, n) for n in (0, 1)],
                9: [lambda n=n: emit_qk_n(2, 0, n) for n in (2, 3)],
            }
            prev = None
            for ci, ch in enumerate(chunks):
                prev = emit_chunk(ch, prev, fill.get(ci, []))
            for jt in range(CSH):
                emit_tail(prev, jt)

    nc.compile()
    return nc


def _get_nc(kc):
    if kc not in _cache:
        _cache[kc] = _build(kc)
    return _cache[kc]


def kernel(hidden_states, attention_mask, Wq, bq, Wk, bk, Wv, bv):
    global last_results
    hidden_states = np.asarray(hidden_states, dtype=np.float32)
    attention_mask = np.asarray(attention_mask, dtype=np.float32)
    Wq = np.asarray(Wq, dtype=np.float32)
    Wk = np.asarray(Wk, dtype=np.float32)
    Wv = np.asarray(Wv, dtype=np.float32)
    bq = np.asarray(bq, dtype=np.float32)
    bk = np.asarray(bk, dtype=np.float32)
    bv = np.asarray(bv, dtype=np.float32)

    any_bias = bool(np.any(bq) or np.any(bk) or np.any(bv))
    kc = 7 if any_bias else 6
    nc = _get_nc(kc)

    in_maps = []
    for c in range(8):
        b = c // 2
        hg = c % 2
        cs = slice(hg * HPC * HEAD_DIM, (hg + 1) * HPC * HEAD_DIM)

        xT = np.zeros((kc * 128, S), dtype=ml_dtypes.bfloat16)
        xT[:HIDDEN] = hidden_states[b].T.astype(ml_dtypes.bfloat16)
        if kc == 7:
            xT[HIDDEN] = 1.0

        def wslice(W, bias):
            w = np.zeros((kc * 128, HPC * HEAD_DIM), dtype=ml_dtypes.bfloat16)
            w[:HIDDEN] = W[:, cs].astype(ml_dtypes.bfloat16)
            if kc == 7:
                w[HIDDEN] = bias[cs].astype(ml_dtypes.bfloat16)
            return w

        em = np.exp(attention_mask[b, 0, 0, :]).astype(np.float32)
        m = {
            "xT": xT,
            "wq": wslice(Wq, bq),
            "wk": wslice(Wk, bk),
            "wv": wslice(Wv, bv),
            "em": np.ascontiguousarray(em.reshape(NJ, 128).T),
        }
        in_maps.append(m)

    res = run_bass_kernel_spmd(
        nc, in_maps, list(range(8)),
        trace=bool(os.environ.get("KERNEL_TRACE")),
    )
    last_results = res

    out = np.empty((B, S, HIDDEN), dtype=np.float32)
    for c in range(8):
        b = c // 2
        hg = c % 2
        r = res.results[c]["out"].astype(np.float32)  # [6, 64, 2048]
        out[b, :, hg * HPC * HEAD_DIM:(hg + 1) * HPC * HEAD_DIM] = (
            r.transpose(2, 0, 1).reshape(S, HPC * HEAD_DIM)
        )
    return out
